# revision 1
# baseline (speedup 1.0000x reference)
"""Multi-head causal attention (B=2, S=2048, D=1024, H=16, hd=64) on 8 trn2 cores.

Sharding: core c handles batch b = c//4 and head-group g = c%4 (heads 4g..4g+4,
d-slice 256g..256g+256 of the QKV projections / Wo rows).  Each core computes a
partial out-projection [2048, 1024]; the host sums the 4 head-group partials per
batch and adds the bias.

Per-core kernel (all matmuls bf16, accumulate f32 in PSUM):
  qT/kT = (x @ Wq/k)^T computed directly as [256, 2048] via lhsT=W chunks.
  v     = x @ Wv in natural [seq, head, 66] layout (col 64 = 1.0 so the
          attention rowsum falls out of the ctx matmul; col 65 = 0 pad).
  S^T   = k_h @ q_h^T  [kpos, qpos] tiles, both heads of a pair concurrently
          via PE row tiling; exp via ACT (scale=1/8) PSUM->SBUF; causal = skip
          invalid column blocks + triangular bf16 mask on diagonal blocks.
  ctx~T = v'_h^T @ expS^T accumulated over kpos blocks -> [66, 512] PSUM
          (row 64 = softmax denominator).
  out  += (ctx~T / rowsum)^T @ Wo rows (normalization: approx reciprocal +
          DRAM-bounce partition broadcast + DVE multiply).

Emission order interleaves independent PE work between dependent attention
steps (PE engine queues are strictly in-order): v proj, pair0 proj, then
attention(pair0) ⟷ proj(pair1), then attention(pair1) ⟷ out-proj chunks.
"""

import sys

import numpy as np

for _p in ("/opt/trn_rl_repo",):
    if _p not in sys.path:
        sys.path.insert(0, _p)

import ml_dtypes

import concourse.bass as bass
import concourse.mybir as mybir
import concourse.tile as tile
from concourse import bacc
from concourse.bass_utils import run_bass_kernel_spmd
from concourse.masks import make_upper_triangular

BF16 = mybir.dt.bfloat16
F32 = mybir.dt.float32

B, S, D, H, HD = 2, 2048, 1024, 16, 64
NCORES = 8
HPC = 4          # heads per core
DHC = HPC * HD   # 256: d-slice per core
P = 128
SB = S // P      # 16 seq blocks
KC = D // P      # 8 contraction chunks for projections
QG = 512         # q column group width
NQG = S // QG    # 4
VW = HD + 2      # 66: v cols per head (64 data + ones + pad; even M for PE)


def _build_body(ctx, tc, io):
    nc = tc.nc
    xT, wq, wk, wv, wo, out = (
        io["xT"], io["wq"], io["wk"], io["wv"], io["wo"], io["out"],
    )

    consts = ctx.enter_context(tc.tile_pool(name="consts", bufs=1))
    persist = ctx.enter_context(tc.tile_pool(name="persist", bufs=1))
    spool = ctx.enter_context(tc.tile_pool(name="spsum", bufs=2, space="PSUM"))
    cxpool = ctx.enter_context(tc.tile_pool(name="cxpsum", bufs=3, space="PSUM"))
    pjpool = ctx.enter_context(tc.tile_pool(name="pjpsum", bufs=1, space="PSUM"))
    espool = ctx.enter_context(tc.tile_pool(name="es", bufs=6))
    nrmpool = ctx.enter_context(tc.tile_pool(name="nrm", bufs=4))
    outpool = ctx.enter_context(tc.tile_pool(name="outsb", bufs=3))
    drampool = ctx.enter_context(tc.tile_pool(name="dram", bufs=1, space="DRAM"))

    # DRAM bounce buffer for partition-broadcasting softmax reciprocals
    rsc = drampool.tile([16, QG], F32, tag="rsc", name="rsc")

    # triangular keep-mask for diagonal blocks: tri[i, j] = 1.0 iff j >= i
    tri = consts.tile([P, P], BF16, tag="tri", name="tri")
    make_upper_triangular(nc, tri[:], val=1.0, diag=True)

    # ---- load inputs: small weights first, then x columns in q-group order
    # so early attention groups unblock as soon as their columns land ----
    xt, wq_sb, wk_sb, wv_sb = [], [], [], []
    named = (("wq", wq, wq_sb), ("wk", wk, wk_sb), ("wv", wv, wv_sb))
    for k in range(KC):
        for name, dram, lst in named:
            t = persist.tile([P, DHC], BF16, tag=f"{name}{k}", name=f"{name}{k}")
            nc.sync.dma_start(out=t[:], in_=dram[k * P:(k + 1) * P, :])
            lst.append(t)
        xt.append(persist.tile([P, S], BF16, tag=f"xt{k}", name=f"xt{k}"))
    wo_sb = []
    for k in range(2):
        t = persist.tile([P, D], BF16, tag=f"wo{k}", name=f"wo{k}")
        nc.sync.dma_start(out=t[:], in_=wo[k * P:(k + 1) * P, :])
        wo_sb.append(t)
    for j in range(NQG):
        for k in range(KC):
            nc.sync.dma_start(out=xt[k][:, j * QG:(j + 1) * QG],
                              in_=xT[k * P:(k + 1) * P, j * QG:(j + 1) * QG])

    # persistent tensors
    v_sb = [persist.tile([P, HPC, VW], BF16, tag=f"v{s}", name=f"v{s}")
            for s in range(SB)]
    qt = [persist.tile([P, S], BF16, tag=f"qt{i}", name=f"qt{i}") for i in range(2)]
    kt = [persist.tile([P, S], BF16, tag=f"kt{i}", name=f"kt{i}") for i in range(2)]
    ctxT = [persist.tile([P, S], BF16, tag=f"ctxT{i}", name=f"ctxT{i}")
            for i in range(2)]

    # ---- emission helpers ----
    def emit_v_proj(sv):
        # two seq blocks (2*sv, 2*sv+1) -> v natural layout
        ps = spool.tile([P, 2, QG], F32, tag="sp", name="sp")
        for par in range(2):
            s = 2 * sv + par
            for k in range(KC):
                nc.tensor.matmul(
                    ps[:, par, 0:DHC],
                    lhsT=xt[k][:, s * P:(s + 1) * P],
                    rhs=wv_sb[k][:],
                    start=(k == 0),
                    stop=(k == KC - 1),
                )
            src_ap = ps[:, par, 0:DHC].rearrange("p (h d) -> p h d", h=HPC)
            nc.vector.tensor_copy(v_sb[s][:, :, 0:HD], src_ap)
            nc.vector.memset(v_sb[s][:, :, HD:VW], 1.0)
            nc.vector.memset(v_sb[s][:, :, HD + 1:VW], 0.0)

    def emit_qk_proj(pair, j):
        # q and k projections for d-chunk `pair`, q column group j
        for w_sb, dst in ((wq_sb, qt), (wk_sb, kt)):
            ps = pjpool.tile([P, QG], F32, tag="pj", name="pj")
            for k in range(KC):
                nc.tensor.matmul(
                    ps[:],
                    lhsT=w_sb[k][:, pair * P:(pair + 1) * P],
                    rhs=xt[k][:, j * QG:(j + 1) * QG],
                    start=(k == 0),
                    stop=(k == KC - 1),
                )
            nc.vector.tensor_copy(dst[pair][:, j * QG:(j + 1) * QG], ps[:])

    def emit_attention_group(pair, g):
        cxs = [cxpool.tile([VW, QG], F32, tag="cx", name="cx") for _ in range(2)]
        nkb = 4 * g + 4
        for kb in range(nkb):
            c0 = P * (kb - 4 * g) if kb >= 4 * g else 0
            sp_t = spool.tile([P, 2, QG], F32, tag="sp", name="sp")
            for hh in range(2):
                nc.tensor.matmul(
                    sp_t[:, hh, c0:QG],
                    lhsT=kt[pair][hh * HD:(hh + 1) * HD, kb * P:(kb + 1) * P],
                    rhs=qt[pair][hh * HD:(hh + 1) * HD, g * QG + c0:(g + 1) * QG],
                    start=True,
                    stop=True,
                )
            es_t = espool.tile([P, 2, QG], BF16, tag="es", name="es")
            nc.scalar.activation(
                es_t[:, :, c0:QG], sp_t[:, :, c0:QG],
                mybir.ActivationFunctionType.Exp, scale=0.125,
            )
            if kb >= 4 * g:
                dst = es_t[:, :, c0:c0 + P]
                t_ap = tri[:]
                tri_b = bass.AP(t_ap.tensor, t_ap.offset,
                                [t_ap.ap[0], [0, 2], t_ap.ap[1]])
                nc.vector.tensor_mul(dst, dst, tri_b)
            for hh in range(2):
                h = 2 * pair + hh
                nc.tensor.matmul(
                    cxs[hh][:, c0:QG],
                    lhsT=v_sb[kb][:, h, :],
                    rhs=es_t[:, hh, c0:QG],
                    start=(kb == 0),
                    stop=(kb == nkb - 1),
                )
        for hh in range(2):
            # custom-DVE ops require base partition 0: stage the rowsum row
            # (psum partition 64) into sbuf with a standard copy first
            rs = nrmpool.tile([1, QG], F32, tag="rs", name="rs")
            nc.vector.tensor_copy(rs[:], cxs[hh][HD:HD + 1, :])
            rc = nrmpool.tile([1, QG], F32, tag="rc", name="rc")
            nc.vector.reciprocal_approx_fast(rc[:], rs[:])
            slot = (pair * NQG + g) * 2 + hh
            nc.sync.dma_start(out=rsc[slot:slot + 1, :], in_=rc[:])
            rb = nrmpool.tile([HD, QG], F32, tag="rb", name="rb")
            sl_ap = rsc[slot:slot + 1, :]
            rc_b = bass.AP(sl_ap.tensor, sl_ap.offset,
                           [[0, HD]] + list(sl_ap.ap[1:]))
            nc.sync.dma_start(out=rb[:], in_=rc_b)
            nc.vector.tensor_mul(
                ctxT[pair][hh * HD:(hh + 1) * HD, g * QG:(g + 1) * QG],
                cxs[hh][0:HD, :],
                rb[:],
            )

    def emit_outproj(m):
        ot = outpool.tile([P, D], F32, tag="ot", name="ot")
        for n2 in range(2):
            ps = pjpool.tile([P, QG], F32, tag="pj", name="pj")
            for kc in range(2):
                nc.tensor.matmul(
                    ps[:],
                    lhsT=ctxT[kc][:, m * P:(m + 1) * P],
                    rhs=wo_sb[kc][:, n2 * QG:(n2 + 1) * QG],
                    start=(kc == 0),
                    stop=(kc == 1),
                )
            if n2 == 0:
                nc.scalar.copy(ot[:, 0:QG], ps[:])
            else:
                nc.vector.tensor_copy(ot[:, QG:D], ps[:])
        nc.sync.dma_start(out=out[m * P:(m + 1) * P, :], in_=ot[:])

    # ---- emission schedule ----
    for sv in range(SB // 2):
        emit_v_proj(sv)
    for j in range(NQG):
        emit_qk_proj(0, j)
    for g in (3, 2, 1, 0):
        emit_attention_group(0, g)
        emit_qk_proj(1, g)
    for g in (3, 2, 1, 0):
        emit_attention_group(1, g)
        for m in range(4 * g, 4 * g + 4):
            emit_outproj(m)


def build_nc():
    from contextlib import ExitStack

    nc = bacc.Bacc()
    io = {
        "xT": nc.dram_tensor("xT", [D, S], BF16, kind="ExternalInput").ap(),
        "wq": nc.dram_tensor("wq", [D, DHC], BF16, kind="ExternalInput").ap(),
        "wk": nc.dram_tensor("wk", [D, DHC], BF16, kind="ExternalInput").ap(),
        "wv": nc.dram_tensor("wv", [D, DHC], BF16, kind="ExternalInput").ap(),
        "wo": nc.dram_tensor("wo", [DHC, D], BF16, kind="ExternalInput").ap(),
        "out": nc.dram_tensor("out", [S, D], F32, kind="ExternalOutput").ap(),
    }
    with tile.TileContext(nc) as tc:
        with ExitStack() as ctx:
            _build_body(ctx, tc, io)
    nc.finalize()
    return nc


_NC = None


def _get_nc():
    global _NC
    if _NC is None:
        _NC = build_nc()
    return _NC


def make_in_maps(x, Wq, Wk, Wv, Wo):
    bf = ml_dtypes.bfloat16
    x = np.asarray(x, dtype=np.float32)
    in_maps = []
    xTs = [np.ascontiguousarray(x[b].T).astype(bf) for b in range(B)]
    for c in range(NCORES):
        b, g = divmod(c, 4)
        sl = slice(DHC * g, DHC * (g + 1))
        in_maps.append({
            "xT": xTs[b],
            "wq": np.ascontiguousarray(np.asarray(Wq, np.float32)[:, sl]).astype(bf),
            "wk": np.ascontiguousarray(np.asarray(Wk, np.float32)[:, sl]).astype(bf),
            "wv": np.ascontiguousarray(np.asarray(Wv, np.float32)[:, sl]).astype(bf),
            "wo": np.ascontiguousarray(np.asarray(Wo, np.float32)[sl, :]).astype(bf),
        })
    return in_maps


def run(in_maps, trace=False, **kw):
    return run_bass_kernel_spmd(_get_nc(), in_maps, list(range(NCORES)),
                                trace=trace, **kw)


def kernel(x, Wq, Wk, Wv, Wo, bo):
    res = run(make_in_maps(x, Wq, Wk, Wv, Wo)).results
    bo = np.asarray(bo, np.float32)
    out = np.empty((B, S, D), np.float32)
    for b in range(B):
        acc = res[4 * b]["out"].astype(np.float32)
        for g in range(1, 4):
            acc = acc + res[4 * b + g]["out"]
        out[b] = acc + bo[None, :]
    return out

